# revision 32
# baseline (speedup 1.0000x reference)
# Trainium2 Bass kernel for CrossAttentionPro:
#   q = x@Wq; k,v = context@Wkv; A = softmax(q k^T / sqrt(d));
#   A = depthwise3x3(A) + conv_b; out = (A @ v) merged @ Wp + bp
#
# Distribution: data-parallel over batch, one batch element per NeuronCore (B=8).
#
# v2 design notes:
#   - Host pre-transposes x/ctx and pre-casts all weights to bf16, so the
#     device runs zero transposes and zero staging casts.
#   - Scores stay transposed: S^T[m,n] = matmul(lhsT=kT[d,m], rhs=qT[d,n]);
#     the two heads of a pair use PE row groups 0/64 and run concurrently.
#   - exp fused on ScalarE (PSUM->SBUF bf16).  Depthwise conv decomposes into
#     3 column-shifted V copies (VA=[up|center] 128 cols, VB=[down|ones] 65
#     cols per head); softmax denominator is the ones column of VB.
#   - 1/den via reciprocal_approx_fast after a PE ones-broadcast.
#   - 9-tap combine on DVE in fp16 with zero-padded Q tiles (even-offset taps
#     hit the 2x DVE mode).  Conv bias is folded into the output-projection
#     bias row: bp2 = bp + biascol^T @ Wp.
#   - Attention pairs are software-pipelined: attend(hp-1) is emitted before
#     scores(hp) so the PE never waits on ScalarE exp.

import os

import numpy as np

B, N, M, C, H = 8, 1024, 1024, 768, 12
D = C // H  # 64
HP = H // 2
NCORES = 8


def build_bass(cfg=None):
    """Builds the single-core Bass program (SPMD across cores via in_maps)."""
    import concourse.bass as bass
    import concourse.mybir as mybir
    import concourse.tile as tile
    from concourse import bacc

    cfg = cfg or {}
    n = cfg.get("N", N)
    m = cfg.get("M", M)
    c = cfg.get("C", C)
    h = cfg.get("H", H)
    d = c // h
    hp_n = h // 2
    assert d == 64 and h % 2 == 0 and n % 128 == 0 and m % 128 == 0 and c % 128 == 0

    fp32 = mybir.dt.float32
    bf16 = mybir.dt.bfloat16
    f16 = mybir.dt.float16
    F = mybir.ActivationFunctionType
    A = mybir.AluOpType
    PSUM = bass.MemorySpace.PSUM

    KT = c // 128      # c tiles
    NT = n // 128      # n (query) tiles
    MT = m // 128      # m (key) tiles
    NHL = n // 512     # n halves for pa/pb psum tiles
    scale = d ** -0.5
    P2 = n + 2         # padded Q width

    nc = bacc.Bacc("TRN2", target_bir_lowering=False, debug=False,
                   num_devices=cfg.get("num_devices", NCORES))

    xT_d = nc.dram_tensor("xT", (c, n), bf16, kind="ExternalInput")
    cT_d = nc.dram_tensor("cT", (c, m), bf16, kind="ExternalInput")
    wq_d = nc.dram_tensor("wq", (c, c), bf16, kind="ExternalInput")
    wkv_d = nc.dram_tensor("wkv", (c, 2 * c), bf16, kind="ExternalInput")
    wp_d = nc.dram_tensor("wp", (c, c), bf16, kind="ExternalInput")
    bp_d = nc.dram_tensor("bp", (1, c), bf16, kind="ExternalInput")
    ident_d = nc.dram_tensor("ident", (128, 128), f16, kind="ExternalInput")
    # wtap[p, hp*9 + 3*i + j] = conv_w[2*hp + p//64, 0, i, j]
    wtap_d = nc.dram_tensor("wtap", (128, 9 * hp_n), fp32, kind="ExternalInput")
    # bvec[p, hp] = conv_b[2*hp + p//64]
    bvec_d = nc.dram_tensor("bvec", (128, hp_n), fp32, kind="ExternalInput")
    out_d = nc.dram_tensor("out", (n, c), fp32, kind="ExternalOutput")

    with tile.TileContext(nc) as tc:
        with tc.tile_pool(name="const", bufs=1) as const, \
             tc.tile_pool(name="persist", bufs=1) as persist:

            wtap = const.tile([128, 9 * hp_n], fp32, name="wtap", tag="wtap")
            nc.gpsimd.dma_start(wtap[:], wtap_d[:])
            bvec = const.tile([128, hp_n], fp32, name="bvec", tag="bvec")
            nc.gpsimd.dma_start(bvec[:], bvec_d[:])
            onescol = const.tile([128, 1], bf16, name="onescol", tag="onescol")
            nc.vector.memset(onescol[:], 1.0)
            onesrow = const.tile([1, 128], bf16, name="onesrow", tag="onesrow")
            nc.vector.memset(onesrow[:], 1.0)
            ones16 = const.tile([1, 128], f16, name="ones16", tag="ones16")
            nc.vector.memset(ones16[:], 1.0)
            bp_sb = const.tile([1, c], bf16, name="bp_sb", tag="bp_sb")
            nc.gpsimd.dma_start(bp_sb[:], bp_d[:])
            biascol = const.tile([128, hp_n], bf16, name="biascol", tag="biascol")
            bp2 = const.tile([1, c], bf16, name="bp2", tag="bp2")
            ident = const.tile([128, 128], f16, name="ident", tag="ident")
            nc.gpsimd.dma_start(ident[:], ident_d[:])
            # diag(w) stationaries for the last pair's PE-side tap combine
            dg = [const.tile([128, 128], f16, name=f"dg{k}", tag=f"dg{k}")
                  for k in range(9)]
            for k in range(9):
                nc.vector.tensor_scalar(dg[k][:], ident[:],
                                        wtap[:, 9 * (hp_n - 1) + k:
                                             9 * (hp_n - 1) + k + 1],
                                        None, op0=A.mult)

            # persistent SBUF tensors
            qT = [persist.tile([128, n], bf16, name=f"qT{i}", tag=f"qT{i}")
                  for i in range(KT)]
            kT = [persist.tile([128, m], bf16, name=f"kT{i}", tag=f"kT{i}")
                  for i in range(KT)]
            VA = [persist.tile([128, 2 * c], bf16, name=f"VA{t}", tag=f"VA{t}")
                  for t in range(MT)]
            VB = [persist.tile([128, 65 * h], bf16, name=f"VB{t}", tag=f"VB{t}")
                  for t in range(MT)]
            aT = [persist.tile([128, n], bf16, name=f"aT{i}", tag=f"aT{i}")
                  for i in range(HP)]
            wp_sb = [persist.tile([128, c], bf16, name=f"wp{k}", tag=f"wp{k}")
                     for k in range(KT)]
            for k in range(KT):
                nc.gpsimd.dma_start(wp_sb[k][:], wp_d[k * 128:(k + 1) * 128, :])

            # ---------------- phase 1: loads + projections ----------------
            with tc.tile_pool(name="ph1", bufs=1) as ph1, \
                 tc.tile_pool(name="dram", bufs=1, space=bass.MemorySpace.DRAM) as dram, \
                 tc.tile_pool(name="pp", bufs=2, space=PSUM) as pp_pool, \
                 tc.tile_pool(name="ps_cs", bufs=2, space=PSUM) as ps_cs, \
                 tc.tile_pool(name="ps_b", bufs=1, space=PSUM) as ps_b:

                xTs = [ph1.tile([128, n], bf16, name=f"xTs{i}", tag=f"xTs{i}")
                       for i in range(KT)]
                cTs = [ph1.tile([128, m], bf16, name=f"cTs{i}", tag=f"cTs{i}")
                       for i in range(KT)]
                wq_sb = [ph1.tile([128, c], bf16, name=f"wq{k}", tag=f"wq{k}")
                         for k in range(KT)]
                wkv_sb = [ph1.tile([128, 2 * c], bf16, name=f"wkv{k}", tag=f"wkv{k}")
                          for k in range(KT)]
                V = [ph1.tile([128, c], bf16, name=f"V{t}", tag=f"V{t}")
                     for t in range(MT)]

                # HAM warmup: dependency-free matmuls keep the PE busy during
                # the input-DMA ramp so the clock gate is at 8/8 when the
                # first projection matmuls issue.
                warm_ps = pp_pool.tile([128, m], fp32, name="pp", tag="pp")
                for _ in range(110):
                    nc.tensor.matmul(warm_ps[:, 0:128], lhsT=ones16[:],
                                     rhs=ones16[:])
                warm_sb = ph1.tile([1, 1], fp32, name="warm_sb", tag="warm_sb")
                nc.vector.tensor_copy(warm_sb[:], warm_ps[0:1, 0:1])

                for k in range(KT):
                    nc.sync.dma_start(cTs[k][:], cT_d[k * 128:(k + 1) * 128, :])
                    nc.gpsimd.dma_start(wkv_sb[k][:],
                                        wkv_d[k * 128:(k + 1) * 128, :])
                for k in range(KT):
                    nc.scalar.dma_start(xTs[k][:], xT_d[k * 128:(k + 1) * 128, :])
                    nc.scalar.dma_start(wq_sb[k][:], wq_d[k * 128:(k + 1) * 128, :])

                def chunks(total, size=512):
                    s = 0
                    while s < total:
                        yield s, min(size, total - s)
                        s += size

                # kT: out[cout 128, m-chunk] = sum_k Wkv[k][:,cout]^T . cTs[k][:, m]
                for co in range(KT):
                    pc = pp_pool.tile([128, m], fp32, name="pp", tag="pp")
                    for (m0, ml) in chunks(m):
                        for k in range(KT):
                            nc.tensor.matmul(
                                pc[:, m0:m0 + ml],
                                lhsT=wkv_sb[k][:, co * 128:(co + 1) * 128],
                                rhs=cTs[k][:, m0:m0 + ml],
                                start=(k == 0), stop=(k == KT - 1))
                    nc.scalar.copy(kT[co][:], pc[:, 0:m])

                # V (natural): out[m-tile 128, c-chunk] = cTs[k][:,m]^T . Wkv[k][:, c+cc]
                for t in range(MT):
                    pv = pp_pool.tile([128, m], fp32, name="pp", tag="pp")
                    for (c0, cl) in chunks(c):
                        for k in range(KT):
                            nc.tensor.matmul(
                                pv[:, c0:c0 + cl],
                                lhsT=cTs[k][:, t * 128:(t + 1) * 128],
                                rhs=wkv_sb[k][:, c + c0:c + c0 + cl],
                                start=(k == 0), stop=(k == KT - 1))
                    nc.vector.tensor_copy(V[t][:], pv[:, 0:c])

                # column sums of V per head pair -> conv bias column
                for hp in range(hp_n):
                    cs = ps_cs.tile([128, 1], fp32, name="cs", tag="cs")
                    for t in range(MT):
                        nc.tensor.matmul(cs[:], lhsT=V[t][:, hp * 128:(hp + 1) * 128],
                                         rhs=onescol[:], start=(t == 0),
                                         stop=(t == MT - 1))
                    nc.vector.tensor_tensor(biascol[:, hp:hp + 1], cs[:],
                                            bvec[:, hp:hp + 1], op=A.mult)

                # bp2 = bp + biascol^T @ Wp   (folds the conv bias into the
                # output projection: rows of out^T get +biascol before @Wp)
                pb2 = ps_b.tile([1, c], fp32, name="pb2", tag="pb2")
                for (c0, cl) in chunks(c):
                    for k in range(KT):
                        nc.tensor.matmul(pb2[:, c0:c0 + cl],
                                         lhsT=biascol[:, k:k + 1],
                                         rhs=wp_sb[k][:, c0:c0 + cl],
                                         start=(k == 0), stop=(k == KT - 1))
                nc.vector.tensor_tensor(bp2[:], pb2[:], bp_sb[:], op=A.add)

                # shifted V copies via a zero-padded DRAM round trip:
                #   VA[t][:, 128h:128h+64]   = V_up (j=0): VA[p] = v[m=128t+p+1]
                #   VA[t][:, 128h+64:128h+128] = V center (j=1)
                #   VB[t][:, 65h:65h+64]     = V_dn (j=2): VB[p] = v[m=128t+p-1]
                #   VB[t][:, 65h+64]         = ones (softmax denominator col)
                def rA(t):
                    return VA[t].rearrange("p (hh x) -> p hh x", x=128)

                def rB(t):
                    return VB[t].rearrange("p (hh x) -> p hh x", x=65)

                vdram = dram.tile([m + 2, c], bf16, name="vdram", tag="vdram")
                zrow = const.tile([1, c], bf16, name="zrow", tag="zrow")
                nc.vector.memset(zrow[:], 0.0)
                nc.sync.dma_start(vdram[0:1, :], zrow[:])
                nc.sync.dma_start(vdram[m + 1:m + 2, :], zrow[:])
                for t in range(MT):
                    nc.sync.dma_start(vdram[t * 128 + 1:(t + 1) * 128 + 1, :], V[t][:])
                for t in range(MT):
                    # v[m = 128t + p + 1]: vdram rows [128t+2 : 128t+130]
                    nc.sync.dma_start(
                        rA(t)[:, :, 0:64],
                        vdram[t * 128 + 2:t * 128 + 130, :]
                        .rearrange("p (hh x) -> p hh x", x=64))
                    # center: vdram rows [128t+1 : 128t+129]
                    nc.sync.dma_start(
                        rA(t)[:, :, 64:128],
                        vdram[t * 128 + 1:t * 128 + 129, :]
                        .rearrange("p (hh x) -> p hh x", x=64))
                    # v[m = 128t + p - 1]: vdram rows [128t : 128t+128]
                    nc.sync.dma_start(
                        rB(t)[:, :, 0:64],
                        vdram[t * 128:t * 128 + 128, :]
                        .rearrange("p (hh x) -> p hh x", x=64))
                    nc.vector.memset(rB(t)[:, :, 64:65], 1.0)

                # qT: out[cout 128, n-chunk] = sum_k Wq[k][:,cout]^T . xTs[k][:, n]
                for co in range(KT):
                    pq = pp_pool.tile([128, n], fp32, name="pp", tag="pp")
                    for (n0, nl) in chunks(n):
                        for k in range(KT):
                            nc.tensor.matmul(
                                pq[:, n0:n0 + nl],
                                lhsT=wq_sb[k][:, co * 128:(co + 1) * 128],
                                rhs=xTs[k][:, n0:n0 + nl],
                                start=(k == 0), stop=(k == KT - 1))
                    nc.scalar.copy(qT[co][:], pq[:, 0:n])

            # ---------------- phase 3: per-head attention ----------------
            with tc.tile_pool(name="es", bufs=2) as es_pool, \
                 tc.tile_pool(name="qpool", bufs=2) as qpool, \
                 tc.tile_pool(name="accpool", bufs=1) as accpool, \
                 tc.tile_pool(name="tmppool", bufs=1) as tmppool, \
                 tc.tile_pool(name="rbcpool", bufs=2) as rbcpool, \
                 tc.tile_pool(name="rrpool", bufs=2) as rrpool, \
                 tc.tile_pool(name="srpool", bufs=2) as srpool, \
                 tc.tile_pool(name="outpool", bufs=2) as outpool, \
                 tc.tile_pool(name="dram3", bufs=4,
                              space=bass.MemorySpace.DRAM) as dram3, \
                 tc.tile_pool(name="ps_s", bufs=2, space=PSUM) as ps_s, \
                 tc.tile_pool(name="ps_pa", bufs=2, space=PSUM) as ps_pa, \
                 tc.tile_pool(name="ps_pb", bufs=2, space=PSUM) as ps_pb:

                def alloc_es():
                    # per-pair exp tile: [p, t, nh, hi, nn]
                    return es_pool.tile([128, MT, 2, 2, 512], bf16,
                                        name="es", tag="es")

                def scores_t(hp, es, t, nh):
                    """Scores + exp for one (m-tile, n-half), both heads of
                    pair hp.  The two heads' K=64 matmuls go to PE row groups
                    0/64 (and separate PSUM banks of one [128,1024] tile) so
                    they run concurrently; one full-width exp covers both."""
                    n0 = nh * 512
                    ss = ps_s.tile([128, 1024], fp32, name="ss", tag="ss")
                    for hi in (0, 1):
                        r0, r1 = hi * 64, (hi + 1) * 64
                        nc.tensor.matmul(
                            ss[:, hi * 512:(hi + 1) * 512],
                            lhsT=kT[hp][r0:r1, t * 128:(t + 1) * 128],
                            rhs=qT[hp][r0:r1, n0:n0 + 512])
                    nc.scalar.activation(es[:, t, nh, :, :], ss[:],
                                         F.Exp, scale=scale)

                def attend_block(hp, es, Q, hi, nh, last=False):
                    """pa/pb accumulation + normalize for one (head, n-half)."""
                    hh = 2 * hp + hi
                    r0, r1 = hi * 64, (hi + 1) * 64
                    n0, nl = nh * 512, 512
                    pa = ps_pa.tile([128, 512], fp32, name="pa", tag="pa")
                    pb = ps_pb.tile([65, 512], fp32, name="pb", tag="pb")
                    for t in range(MT):
                        nc.tensor.matmul(pa[:, 0:nl],
                                         lhsT=VA[t][:, 128 * hh:128 * (hh + 1)],
                                         rhs=es[:, t, nh, hi, :],
                                         start=(t == 0), stop=(t == MT - 1))
                    for t in range(MT):
                        nc.tensor.matmul(pb[:, 0:nl],
                                         lhsT=VB[t][:, 65 * hh:65 * (hh + 1)],
                                         rhs=es[:, t, nh, hi, :],
                                         start=(t == 0), stop=(t == MT - 1))
                    rbc = rbcpool.tile([128, 512], fp32, name="rbc", tag="rbc")
                    if last:
                        # low-latency den path for the final pair: ones
                        # outer-product broadcast on the PE (the ss PSUM pool
                        # is idle here), then fast reciprocal.
                        s16 = srpool.tile([1, 512], f16, name="srow16",
                                          tag="srow")
                        nc.vector.tensor_copy(s16[:, 0:nl], pb[64:65, 0:nl])
                        bc = ps_s.tile([128, 1024], fp32, name="ss", tag="ss")
                        nc.tensor.matmul(bc[:, 0:nl], lhsT=ones16[:],
                                         rhs=s16[:, 0:nl])
                        nc.vector.reciprocal_approx_fast(rbc[:, 0:nl],
                                                         bc[:, 0:nl])
                    else:
                        # steady state: 1/den on the [1,512] row, broadcast
                        # to 128 partitions with a stride-0 DMA read through
                        # a DRAM staging row (no PE/PSUM involved).
                        srow = srpool.tile([1, 512], fp32, name="srow",
                                           tag="srow")
                        nc.scalar.copy(srow[:, 0:nl], pb[64:65, 0:nl])
                        rrow = rrpool.tile([1, 512], fp32, name="rrow",
                                           tag="rrow")
                        nc.vector.reciprocal_approx_fast(rrow[:, 0:nl],
                                                         srow[:, 0:nl])
                        rd = dram3.tile([1, 512], fp32, name="rd", tag="rd")
                        nc.sync.dma_start(rd[:], rrow[:])
                        nc.sync.dma_start(rbc[:],
                                          rd[0:1, :].broadcast_to([128, 512]))
                    # normalized Q tiles in padded fp16 layout
                    dst = slice(1 + n0, 1 + n0 + nl)
                    nc.vector.tensor_tensor(Q[0][r0:r1, dst], pa[0:64, 0:nl],
                                            rbc[0:64, 0:nl], op=A.mult)
                    nc.vector.tensor_tensor(Q[1][r0:r1, dst], pa[64:128, 0:nl],
                                            rbc[64:128, 0:nl], op=A.mult)
                    nc.vector.tensor_tensor(Q[2][r0:r1, dst], pb[0:64, 0:nl],
                                            rbc[0:64, 0:nl], op=A.mult)

                def taps(hp, Q, half=None):
                    # 9-tap combine: aT[p,nn] = sum_ij w[i,j]*Q_j[p, nn+i-1]
                    # Q padded with zero cols at 0 and P2-1; tap (i,j) reads
                    # Q[j][:, i:i+n].  tensor_scalar has fast DVE uops (stt
                    # does not), so each tap is ts (2x/4x) + tensor_tensor
                    # add (2x) — both on DVE (GpSimd elementwise is ~30x
                    # slower and starves the shared SBUF port).
                    # half=0/1 restricts the combine to one n-half (used for
                    # the last pair so phase 4 can start on the ready half).
                    if half is None:
                        lo, ln = 0, n
                    else:
                        lo, ln = half * 512, 512
                    acc = accpool.tile([128, n], f16, name="acc", tag="acc")

                    def wv(i, j):
                        idx = hp * 9 + 3 * i + j
                        return wtap[:, idx:idx + 1]

                    nc.vector.tensor_scalar(acc[:, lo:lo + ln],
                                            Q[1][:, 1 + lo:1 + lo + ln],
                                            wv(1, 1), None, op0=A.mult)
                    for (i, j) in ((0, 0), (0, 1), (0, 2), (2, 0), (2, 1),
                                   (2, 2), (1, 0)):
                        tmp = tmppool.tile([128, n], f16, name="tmp", tag="tmp")
                        nc.vector.tensor_scalar(tmp[:, lo:lo + ln],
                                                Q[j][:, i + lo:i + lo + ln],
                                                wv(i, j), None, op0=A.mult)
                        nc.vector.tensor_tensor(acc[:, lo:lo + ln],
                                                tmp[:, lo:lo + ln],
                                                acc[:, lo:lo + ln], op=A.add)
                    tmp = tmppool.tile([128, n], f16, name="tmp", tag="tmp")
                    nc.vector.tensor_scalar(tmp[:, lo:lo + ln],
                                            Q[2][:, 1 + lo:1 + lo + ln],
                                            wv(1, 2), None, op0=A.mult)
                    nc.vector.tensor_tensor(aT[hp][:, lo:lo + ln],
                                            tmp[:, lo:lo + ln],
                                            acc[:, lo:lo + ln], op=A.add)

                def proj_out(t):
                    # output projection for one n-tile, PSUM from the (now
                    # idle) ss pool so no pool-transition barrier is paid.
                    pf = ps_s.tile([128, 1024], fp32, name="ss", tag="ss")
                    for (c0, cl) in chunks(c):
                        for k in range(KT):
                            nc.tensor.matmul(pf[:, c0:c0 + cl],
                                             lhsT=aT[k][:, t * 128:(t + 1) * 128],
                                             rhs=wp_sb[k][:, c0:c0 + cl],
                                             start=(k == 0), stop=False)
                        nc.tensor.matmul(pf[:, c0:c0 + cl], lhsT=onesrow[:],
                                         rhs=bp2[:, c0:c0 + cl], start=False,
                                         stop=True)
                    ot = outpool.tile([128, c], fp32, name="ot", tag="ot")
                    nc.scalar.copy(ot[:], pf[:, 0:c])
                    nc.scalar.dma_start(out_d[t * 128:(t + 1) * 128, :], ot[:])

                # Software pipeline, 1-deep: iteration hp runs attend(hp)
                # interleaved with scores(hp+1).  Scores are emitted nh-major
                # and attend blocks are ordered nh-major too, so each attend
                # block's es quarter was produced a full iteration earlier.
                quarters = [(0, range(0, 4)), (0, range(4, 8)),
                            (1, range(0, 4)), (1, range(4, 8))]
                es_l = [alloc_es()]
                for (nh, ts_r) in quarters:
                    for t in ts_r:
                        scores_t(0, es_l[0], t, nh)
                for hp in range(hp_n):
                    hp_s = hp + 1
                    last = hp == hp_n - 1
                    if not last:
                        es_l.append(alloc_es())
                    Q = [qpool.tile([128, P2], f16, name=f"Q{j}", tag=f"Q{j}")
                         for j in range(3)]
                    for j in range(3):
                        nc.vector.memset(Q[j][:, 0:1], 0.0)
                        nc.vector.memset(Q[j][:, P2 - 1:P2], 0.0)
                    for b, (hi, nh) in enumerate(
                            ((0, 0), (1, 0), (0, 1), (1, 1))):
                        attend_block(hp, es_l[hp], Q, hi, nh, last=last)
                        if not last:
                            qnh, qts = quarters[b]
                            for t in qts:
                                scores_t(hp_s, es_l[hp_s], t, qnh)
                    if not last:
                        taps(hp, Q)
                    else:
                        # last pair: run the 9-tap combine on the PE with
                        # diag(w) stationaries and shifted rhs windows (the
                        # zero pad columns make every tap a full-window
                        # accumulate), so the tail stays on the busy engine.
                        # Chunk-major so each half's aT copy and the first
                        # output-projection tiles overlap the other half.
                        acc_ps = ps_s.tile([128, 1024], fp32, name="ss",
                                           tag="ss")
                        order = ((1, 1), (0, 0), (0, 1), (0, 2), (2, 0),
                                 (2, 1), (2, 2), (1, 0), (1, 2))
                        for (n0, nl) in chunks(n):
                            for x, (i, j) in enumerate(order):
                                nc.tensor.matmul(
                                    acc_ps[:, n0:n0 + nl],
                                    lhsT=dg[3 * i + j][:],
                                    rhs=Q[j][:, i + n0:i + n0 + nl],
                                    start=(x == 0), stop=(x == len(order) - 1))
                            nc.scalar.copy(aT[hp][:, n0:n0 + nl],
                                           acc_ps[:, n0:n0 + nl])
                            if n0 == 0:
                                continue
                        for t in range(NT):
                            proj_out(t)

    nc.compile()
    return nc


def chunks(total, size=512):
    s = 0
    while s < total:
        yield s, min(size, total - s)
        s += size


def make_host_inputs(x, context, Wq, Wkv, conv_w, conv_b, Wp, bp, cfg=None):
    import ml_dtypes

    bf16 = ml_dtypes.bfloat16
    cfg = cfg or {}
    h = cfg.get("H", H)
    hp_n = h // 2
    wtap = np.empty((128, 9 * hp_n), np.float32)
    bvec = np.empty((128, hp_n), np.float32)
    for hp in range(hp_n):
        for p in range(128):
            head = 2 * hp + p // 64
            bvec[p, hp] = conv_b[head]
            for i in range(3):
                for j in range(3):
                    wtap[p, hp * 9 + 3 * i + j] = conv_w[head, 0, i, j]
    shared = {
        "ident": np.eye(128, dtype=np.float16),
        "wq": np.ascontiguousarray(Wq).astype(bf16),
        "wkv": np.ascontiguousarray(Wkv).astype(bf16),
        "wp": np.ascontiguousarray(Wp).astype(bf16),
        "bp": np.ascontiguousarray(bp).reshape(1, -1).astype(bf16),
        "wtap": wtap,
        "bvec": bvec,
    }
    in_maps = []
    for b in range(x.shape[0]):
        im = dict(shared)
        im["xT"] = np.ascontiguousarray(x[b].T).astype(bf16)
        im["cT"] = np.ascontiguousarray(context[b].T).astype(bf16)
        in_maps.append(im)
    return in_maps


def kernel(x, context, Wq, Wkv, conv_w, conv_b, Wp, bp):
    from concourse.bass_utils import run_bass_kernel_spmd

    x = np.asarray(x, np.float32)
    context = np.asarray(context, np.float32)
    Wq = np.asarray(Wq, np.float32)
    Wkv = np.asarray(Wkv, np.float32)
    conv_w = np.asarray(conv_w, np.float32)
    conv_b = np.asarray(conv_b, np.float32)
    Wp = np.asarray(Wp, np.float32)
    bp = np.asarray(bp, np.float32)

    nc = build_bass()
    in_maps = make_host_inputs(x, context, Wq, Wkv, conv_w, conv_b, Wp, bp)
    res = run_bass_kernel_spmd(nc, in_maps, core_ids=list(range(NCORES)),
                               trace=bool(int(os.environ.get("KERNEL_TRACE", "0"))))
    out = np.stack([r["out"] for r in res.results], axis=0)
    if res.exec_time_ns is not None:
        print(f"HW exec time: {res.exec_time_ns} ns")
    kernel.last_result = res
    return out
